# revision 33
# baseline (speedup 1.0000x reference)
"""CFD-GCN Trainium2 kernel: 6-layer GCN + KNN-interpolate on 8 NeuronCores.

v3 strategy (node sharding, feature-major residency, bf16 message path,
part-split AllGather hidden behind part-split gathers):
  - Fine nodes sharded 6250/core (padded to 6272 = 49*128 rows); dest
    tiles processed in groups of G_TILES=4 per gather window.
  - Nodes are additionally split into NPARTS=3 tile-range parts. Each
    layer's z is AllGathered per part (bf16); the per-edge gathers are
    split by SOURCE part, so part-1/2 gathers run while part-3's AG is
    still in flight -> the AG mostly leaves the critical path.
  - Per merged layer loop, per dest tile: scatter-add via one-hot matmuls
    (bf16 S from static iota on DVE) into PSUM [feat, dest], bias+relu on
    ScalarE -> hT (feature-major), then immediately the NEXT layer's dense
    z = h W (bf16, PE) -> zsh part tensors, so AG parts fire mid-loop.
  - Gathers: SWDGE dma_gather, 1KB bf16 rows (descriptor-rate-bound, so
    full 512-wide rows beat chunked), 4 SWDGE queues round-robin, calls
    capped at 7 blocks (896 descs < 1024-desc ring).
  - Edge slots are packed contiguously per (group, part) (shared boundary
    blocks, masked via ecol=-1 sentinel columns per consuming tile);
    padding only at segment ends (dummy index 0, norm 0) so a static
    num_idxs covers each call (no count registers).
  - First layer sparse-first on the replicated 6-wide input (no AG); last
    layer dense-first (A (h W5), 3-wide, 128-padded rows).
  - KNN-interpolate (f32 distance path - cancellation-sensitive) tiles
    interleaved across the first three merged loops.
"""

import math
import numpy as np

# ---------------------------------------------------------------- constants
N_FINE = 50000
N_COARSE = 2000
HID = 512
OUT = 3
NCORES = 8
P = 128
G_TILES = 4          # dest tiles per gather window
NPARTS = 3           # node-range parts for AG pipelining
MAX_CALL_BLOCKS = 7  # <=896 descriptors per dma_gather call (ring is 1024)
NQUEUES = 4

_PROGRAM_CACHE = {}


# ---------------------------------------------------------------- host side
def _wrap16(flat):
    """int16 index list -> dma_gather layout [P, len/16] (wrapped in 16
    partitions, replicated across the 8 Q7 cores)."""
    L = len(flat) // 16
    w = np.asarray(flat, np.int16).reshape(L, 16).T  # [16, L]
    return np.tile(w, (P // 16, 1))


def _make_parts(nt):
    np_eff = min(NPARTS, nt)
    base_sz = nt // np_eff
    rem = nt - base_sz * np_eff
    part_tiles = []
    t0 = 0
    for pi in range(np_eff):
        sz = base_sz + (1 if pi < rem else 0)
        part_tiles.append((t0, t0 + sz))
        t0 += sz
    return part_tiles


def _part_of(t, part_tiles):
    for pi, (t0, t1) in enumerate(part_tiles):
        if t0 <= t < t1:
            return pi
    raise ValueError(t)


def _preprocess_edges(edge_index, n_fine, ncores):
    """See module docstring: per (dest-group, source-part) contiguous slot
    packing with shared boundary blocks and per-tile masked ecol columns."""
    nsh = n_fine // ncores
    nt = math.ceil(nsh / P)
    padsh = nt * P

    part_tiles = _make_parts(nt)
    nparts = len(part_tiles)
    part_rows = [(t1 - t0) * P for t0, t1 in part_tiles]
    part_rows_full = [r * ncores for r in part_rows]

    row = np.asarray(edge_index[0]).astype(np.int64)
    col = np.asarray(edge_index[1]).astype(np.int64)
    loop = np.arange(n_fine, dtype=np.int64)
    row = np.concatenate([row, loop])
    col = np.concatenate([col, loop])

    deg = np.bincount(col, minlength=n_fine).astype(np.float32)
    dis = 1.0 / np.sqrt(deg)
    normv = (dis[row] * dis[col]).astype(np.float32)

    # source -> (part, row within part table, AG rank-major layout)
    s_core = row // nsh
    s_loc = row % nsh
    s_tile = s_loc // P
    s_part = np.zeros_like(row)
    s_prow = np.zeros_like(row)
    for pi, (pt0, pt1) in enumerate(part_tiles):
        m = (s_tile >= pt0) & (s_tile < pt1)
        s_part[m] = pi
        s_prow[m] = s_core[m] * (pt1 - pt0) * P + (s_loc[m] - pt0 * P)

    order = np.argsort(col, kind="stable")
    col_s = col[order]
    norm_s = normv[order]
    spart_s = s_part[order]
    sprow_s = s_prow[order]

    groups = []
    for g0 in range(0, nt, G_TILES):
        groups.append(list(range(g0, min(nt, g0 + G_TILES))))

    # per (core, group, part): filtered edge segment; K = max-core blocks
    seg = {}
    K = [[0] * nparts for _ in groups]
    for c in range(ncores):
        base = c * nsh
        for gi_i, tiles in enumerate(groups):
            lo = base + tiles[0] * P
            hi = min(base + (tiles[-1] + 1) * P, base + nsh)
            a = np.searchsorted(col_s, lo, "left")
            b = np.searchsorted(col_s, hi, "left")
            for pi in range(nparts):
                m = spart_s[a:b] == pi
                seg[(c, gi_i, pi)] = (a, b, m)
                K[gi_i][pi] = max(K[gi_i][pi], math.ceil(int(m.sum()) / P))

    grp_info = []
    slot_off = 0
    for gi_i, tiles in enumerate(groups):
        parts = []
        mb = 0
        for pi in range(nparts):
            kb = K[gi_i][pi]
            parts.append({"k": kb, "mb": mb, "slot_off": slot_off})
            mb += kb
            slot_off += kb * P
        grp_info.append({"tiles": tiles, "parts": parts, "nblk": mb})
    slots_tot = slot_off
    maxblk_g = max(gi["nblk"] for gi in grp_info)

    # per-tile static block ranges (union over cores)
    tile_ranges = {}
    for c in range(ncores):
        for gi_i, tiles in enumerate(groups):
            for pi in range(nparts):
                a, b, m = seg[(c, gi_i, pi)]
                sub_cols = col_s[a:b][m]
                for t in tiles:
                    lo_t = np.searchsorted(sub_cols, c * nsh + t * P, "left")
                    hi_t = np.searchsorted(sub_cols, c * nsh + (t + 1) * P, "left")
                    if hi_t == lo_t:
                        continue
                    b0 = lo_t // P
                    b1 = (hi_t + P - 1) // P
                    key = (gi_i, t, pi)
                    if key in tile_ranges:
                        ob0, ob1 = tile_ranges[key]
                        tile_ranges[key] = (min(ob0, b0), max(ob1, b1))
                    else:
                        tile_ranges[key] = (b0, b1)
    for gi_i, tiles in enumerate(groups):
        for t in tiles:
            for pi in range(nparts):
                tile_ranges.setdefault((gi_i, t, pi), (0, 0))

    colid = {}
    ncols = 0
    for gi_i, tiles in enumerate(groups):
        for t in tiles:
            for pi in range(nparts):
                b0, b1 = tile_ranges[(gi_i, t, pi)]
                for bb in range(b0, b1):
                    colid[(gi_i, t, pi, bb)] = ncols
                    ncols += 1

    out = []
    for c in range(ncores):
        idx = np.zeros(slots_tot, np.int64)
        ecol = np.full((P, ncols), -1.0, np.float32)
        enorm = np.zeros((P, ncols), np.float32)
        for gi_i, tiles in enumerate(groups):
            for pi in range(nparts):
                a, b, m = seg[(c, gi_i, pi)]
                so = grp_info[gi_i]["parts"][pi]["slot_off"]
                rows_p = sprow_s[a:b][m]
                cols_p = col_s[a:b][m]
                nrm_p = norm_s[a:b][m]
                n = len(rows_p)
                idx[so:so + n] = rows_p
                for t in tiles:
                    lo_t = np.searchsorted(cols_p, c * nsh + t * P, "left")
                    hi_t = np.searchsorted(cols_p, c * nsh + (t + 1) * P, "left")
                    if hi_t == lo_t:
                        continue
                    s = np.arange(lo_t, hi_t)
                    bb = s // P
                    pp = s % P
                    cid = np.array([colid[(gi_i, t, pi, int(x))] for x in bb])
                    ecol[pp, cid] = (cols_p[lo_t:hi_t]
                                     - (c * nsh + t * P)).astype(np.float32)
                    enorm[pp, cid] = nrm_p[lo_t:hi_t]
        # Pre-expand the one-hot scatter matrices (layer-invariant): for
        # column cid, S[pp, dest] = enorm[pp, cid] iff ecol[pp, cid] == dest.
        import ml_dtypes
        sblk = np.zeros((P, ncols * P), np.float32)
        ppi, cidi = np.nonzero(ecol >= 0.0)
        dest = ecol[ppi, cidi].astype(np.int64)
        sblk[ppi, cidi * P + dest] = enorm[ppi, cidi]
        out.append({"idx": _wrap16(idx),
                    "sblk": sblk.astype(ml_dtypes.bfloat16)})

    # per-group contiguous colid ranges (colid assigned gi-major)
    grp_cols = []
    for gi_i, tiles in enumerate(groups):
        ids = [colid[(gi_i, t, pi, bb)]
               for t in tiles for pi in range(nparts)
               for bb in range(*tile_ranges[(gi_i, t, pi)])]
        grp_cols.append((min(ids), len(ids)) if ids else (0, 0))

    layout = {
        "groups": groups, "grp_info": grp_info, "K": K,
        "grp_cols": grp_cols,
        "tile_ranges": tile_ranges, "colid": colid, "ncols": ncols,
        "slots_tot": slots_tot, "maxblk_g": maxblk_g,
        "part_tiles": part_tiles, "part_rows": part_rows,
        "part_rows_full": part_rows_full,
        "Kkey": tuple(tuple(k) for k in K),
        "Rkey": tuple(sorted(tile_ranges.items())),
    }
    return nt, padsh, layout, out


def _part_layout_full(x, nsh, ncores, part_tiles):
    """Full array [n_fine, d] -> per part [ncores*part_rows, d] in AG output
    layout (rank-major, local tile order), zero-padded."""
    d = x.shape[1]
    outs = []
    for (t0, t1) in part_tiles:
        pr = (t1 - t0) * P
        o = np.zeros((ncores * pr, d), x.dtype)
        for c in range(ncores):
            lo = c * nsh + t0 * P
            hi = min(c * nsh + t1 * P, (c + 1) * nsh)
            if hi > lo:
                o[c * pr: c * pr + (hi - lo)] = x[lo:hi]
        outs.append(o)
    return outs


# ---------------------------------------------------------------- device side
def build_program(n_fine, n_coarse, hid, out_dim, ncores, nt, layout):
    import concourse.bass as bass
    import concourse.mybir as mybir
    from concourse.bacc import Bacc
    from concourse.tile import TileContext
    from concourse.masks import make_identity
    from contextlib import ExitStack

    F32 = mybir.dt.float32
    BF16 = mybir.dt.bfloat16
    I32 = mybir.dt.int32
    I16 = mybir.dt.int16
    padsh = nt * P
    kc = hid // P
    rg = [list(range(ncores))]
    AF = mybir.ActivationFunctionType
    ALU = mybir.AluOpType
    IOO = bass.IndirectOffsetOnAxis
    ncpad = math.ceil(n_coarse / 512) * 512
    ncc = math.ceil(n_coarse / 512)

    groups = layout["groups"]
    grp_info = layout["grp_info"]
    grp_cols = layout["grp_cols"]
    maxcols_g = max(c for _, c in grp_cols)
    tile_ranges = layout["tile_ranges"]
    colid = layout["colid"]
    ncols = layout["ncols"]
    slots_tot = layout["slots_tot"]
    maxblk_g = layout["maxblk_g"]
    part_tiles = layout["part_tiles"]
    part_rows = layout["part_rows"]
    part_rows_full = layout["part_rows_full"]
    nparts = len(part_tiles)

    nc = Bacc(num_devices=ncores, num_swdge_queues=NQUEUES)

    # ---- kernel I/O (per core) ----
    h0p = [nc.declare_dram_parameter(f"h0p{pi}", [part_rows_full[pi], 128],
                                     BF16, isOutput=False)
           for pi in range(nparts)]
    idx = nc.declare_dram_parameter("idx", [P, slots_tot // 16], I16, isOutput=False)
    sblk = nc.declare_dram_parameter("sblk", [P, ncols * P], BF16, isOutput=False)
    xposT = nc.declare_dram_parameter("xposT", [2, padsh], F32, isOutput=False)
    xpos_nm = nc.declare_dram_parameter("xpos_nm", [padsh, 2], F32, isOutput=False)
    coarseT = nc.declare_dram_parameter("coarseT", [2, n_coarse], F32, isOutput=False)
    ycoarse = nc.declare_dram_parameter("ycoarse", [n_coarse, out_dim], F32, isOutput=False)
    w_mid = [nc.declare_dram_parameter(n, [hid, hid], BF16, isOutput=False)
             for n in ("w1", "w2", "we0", "we1")]
    b_mid = [nc.declare_dram_parameter(n, [hid], F32, isOutput=False)
             for n in ("b1", "b2", "be0", "be1")]
    w0 = nc.declare_dram_parameter("w0", [6, hid], BF16, isOutput=False)
    b0 = nc.declare_dram_parameter("b0", [hid], F32, isOutput=False)
    wtop = nc.declare_dram_parameter("wtop", [out_dim, hid], BF16, isOutput=False)
    w5 = nc.declare_dram_parameter("w5", [hid, out_dim], BF16, isOutput=False)
    b5 = nc.declare_dram_parameter("b5", [out_dim], F32, isOutput=False)
    y_out = nc.declare_dram_parameter("out", [padsh, out_dim], F32, isOutput=True)

    # ---- internal DRAM: per mid-layer, per part ----
    zsh = [[nc.dram_tensor(f"zsh{i}_{pi}", [part_rows[pi], hid], BF16)
            for pi in range(nparts)] for i in range(4)]
    zfull = [[nc.dram_tensor(f"zfull{i}_{pi}", [part_rows_full[pi], hid], BF16,
                             addr_space="Shared") for pi in range(nparts)]
             for i in range(4)]
    z5sh = [nc.dram_tensor(f"z5sh_{pi}", [part_rows[pi], 128], BF16)
            for pi in range(nparts)]
    z5full = [nc.dram_tensor(f"z5full_{pi}", [part_rows_full[pi], 128], BF16,
                             addr_space="Shared") for pi in range(nparts)]

    LANE2Q = (0, 1, 2, 3, 0, 1, 2, 3)
    pctr = [0]            # mirrors Tile's 8-lane DMASW rotation
    pend_ind = []         # deferred q0-pinned indirect emissions (callables)

    with TileContext(nc) as tc:
        with ExitStack() as ctx:
            main = ctx.enter_context(tc.tile_pool(name="main", bufs=1))
            wpool = ctx.enter_context(tc.tile_pool(name="wpool", bufs=2))
            spool = ctx.enter_context(tc.tile_pool(name="spool", bufs=2))
            zp = ctx.enter_context(tc.tile_pool(name="zp", bufs=2))
            smallp = ctx.enter_context(tc.tile_pool(name="smallp", bufs=2))
            msgp = ctx.enter_context(tc.tile_pool(name="msgp", bufs=2))
            msg6p = ctx.enter_context(tc.tile_pool(name="msg6p", bufs=2))
            knn_k = ctx.enter_context(tc.tile_pool(name="knn", bufs=1))
            ppA = ctx.enter_context(tc.tile_pool(name="ppA", bufs=2, space="PSUM"))
            ppB = ctx.enter_context(tc.tile_pool(name="ppB", bufs=2, space="PSUM"))
            ppC = ctx.enter_context(tc.tile_pool(name="ppC", bufs=2, space="PSUM"))

            def accps():
                return ppA.tile([P, hid], F32, tag="acc", name="acc")

            def densps():
                return ppB.tile([P, hid], F32, tag="dacc", name="dacc")

            def tps():
                return ppC.tile([P, 512], F32, tag="tp", name="tp")

            # ---------- persistent tiles ----------
            hT = main.tile([P, kc, padsh], BF16, tag="hT")
            y3n = main.tile([P, nt, out_dim], F32, tag="y3n")
            iden32 = main.tile([P, P], F32, tag="iden32")
            idx_sb = main.tile([P, slots_tot // 16], I16, tag="idx_sb")
            wtop_sb = main.tile([out_dim, hid], BF16, tag="wtop_sb")

            nc.sync.dma_start(out=idx_sb[:], in_=idx[:, :])
            nc.sync.dma_start(out=wtop_sb[:], in_=wtop[:, :])

            make_identity(nc, iden32[:])

            # ---------- helpers ----------
            def load_w_mid(wd):
                w_sb = wpool.tile([P, kc, hid], BF16, tag="w_sb")
                nc.sync.dma_start(
                    out=w_sb[:], in_=wd[:, :].rearrange("(k p) h -> p k h", p=P))
                return w_sb

            def load_b_mid(bd):
                b_sb = wpool.tile([P, kc], F32, tag="b_sb")
                nc.sync.dma_start(out=b_sb[:], in_=bd[:].rearrange("(k p) -> p k", p=P))
                return b_sb

            def load_S(gi_i):
                """DMA the group's pre-expanded S strip; returns (tile, c0)."""
                c0, cnt = grp_cols[gi_i]
                s_sb = spool.tile([P, maxcols_g * P], BF16, tag="s_sb")
                if cnt:
                    nc.sync.dma_start(out=s_sb[:, :cnt * P],
                                      in_=sblk[:, c0 * P:(c0 + cnt) * P])
                return s_sb, c0

            dummyp = ctx.enter_context(tc.tile_pool(name="dummyp", bufs=2))

            def emit_pending_q0():
                """If the next DMASW lane maps to queue 0, drain one pending
                q0-pinned indirect DMA emission (keeps sem<->queue binding
                consistent)."""
                while pend_ind and LANE2Q[pctr[0] % 8] == 0:
                    fn, n_dmas = pend_ind.pop(0)
                    fn()
                    pctr[0] += n_dmas

            def dummy_gather():
                d = dummyp.tile([P, 1, 128], BF16, tag="dummy")
                nc.gpsimd.dma_gather(d[:], h0p[0][:, :], idx_sb[:, 0:1],
                                     16, 16, 128,
                                     queue_num=LANE2Q[pctr[0] % 8])
                pctr[0] += 1

            def flush_pending_q0():
                while pend_ind:
                    if LANE2Q[pctr[0] % 8] == 0:
                        emit_pending_q0()
                    else:
                        dummy_gather()

            def gather_group(gi_i, msg, elem, tabs):
                """Per part: dma_gather calls (split at MAX_CALL_BLOCKS),
                queue = lane map (keeps sem<->queue binding consistent)."""
                gi = grp_info[gi_i]
                for pi in range(nparts):
                    pinfo = gi["parts"][pi]
                    kb, mb, so = pinfo["k"], pinfo["mb"], pinfo["slot_off"]
                    b = 0
                    while b < kb:
                        emit_pending_q0()
                        bn = min(MAX_CALL_BLOCKS, kb - b)
                        q = LANE2Q[pctr[0] % 8]
                        pctr[0] += 1
                        nc.gpsimd.dma_gather(
                            msg[:, mb + b: mb + b + bn, :], tabs[pi][:, :],
                            idx_sb[:, (so + b * P) // 16: (so + (b + bn) * P) // 16],
                            bn * P, bn * P, elem, queue_num=q,
                            single_packet=(bn * P <= 1024))
                        b += bn

            def tile_blocks(gi_i, t):
                """(msg_block_index, ecol_column) pairs for tile t."""
                gi = grp_info[gi_i]
                res = []
                for pi in range(nparts):
                    b0, b1 = tile_ranges[(gi_i, t, pi)]
                    mb = gi["parts"][pi]["mb"]
                    for bb in range(b0, b1):
                        res.append((mb + bb, colid[(gi_i, t, pi, bb)]))
                return res

            # ---------- KNN prep (f32: cancellation-sensitive) ----------
            coarse3 = main.tile([3, ncpad], F32, tag="coarse3")
            fsqneg = main.tile([P, nt], F32, tag="fsqneg")
            idx_buf = main.tile([P, nt, 8], mybir.dt.uint32, tag="idx_buf")
            idx3 = main.tile([P, nt, 3], mybir.dt.uint32, tag="idx3")
            wv_buf = main.tile([P, nt, 3], F32, tag="wv_buf")
            mones_sb = main.tile([1, P], F32, tag="mones_sb")
            nc.vector.memset(mones_sb[:], -1.0)
            with tc.tile_pool(name="knnprep", bufs=1) as kprep:
                nc.sync.dma_start(out=coarse3[0:2, 0:n_coarse],
                                  in_=coarseT[:, :])
                pones = kprep.tile([2, 1], F32, tag="pones")
                nc.vector.memset(pones[:], 1.0)
                for i in range(ncc):
                    a, b = i * 512, min((i + 1) * 512, n_coarse)
                    sqc = kprep.tile([2, 512], F32, tag="sqc")
                    nc.vector.tensor_tensor(out=sqc[:, : b - a],
                                            in0=coarse3[0:2, a:b],
                                            in1=coarse3[0:2, a:b], op=ALU.mult)
                    ps = tps()
                    nc.tensor.matmul(out=ps[0:1, : b - a], lhsT=pones[:],
                                     rhs=sqc[:, : b - a], start=True, stop=True)
                    csq_c = kprep.tile([1, 512], F32, tag="csq_c")
                    nc.vector.tensor_copy(out=csq_c[:, : b - a],
                                          in_=ps[0:1, : b - a])
                    nc.sync.dma_start(out=coarse3[2:3, a:b],
                                      in_=csq_c[:, : b - a])

                xnm = kprep.tile([P, nt, 2], F32, tag="xnm")
                nc.sync.dma_start(
                    out=xnm[:], in_=xpos_nm[:, :].rearrange("(t p) d -> p t d", p=P))
                sqn = kprep.tile([P, nt, 2], F32, tag="sqn")
                nc.vector.tensor_tensor(out=sqn[:], in0=xnm[:], in1=xnm[:],
                                        op=ALU.mult)
                nc.vector.tensor_reduce(out=fsqneg[:], in_=sqn[:],
                                        axis=mybir.AxisListType.X, op=ALU.add,
                                        negate=True)

            def knn_phase1(t):
                """Phase 1: d2, top-3, weights -> idx_buf/wv_buf."""
                tp_ = t * P
                xp_t = knn_k.tile([2, P], F32, tag="xp_t")
                nc.sync.dma_start(out=xp_t[:], in_=xposT[:, tp_:tp_ + P])
                lhsT3 = knn_k.tile([3, P], F32, tag="lhsT3")
                nc.vector.tensor_scalar_mul(lhsT3[0:2, :], xp_t[:], 2.0)
                nc.sync.dma_start(out=lhsT3[2:3, :], in_=mones_sb[:])

                d2 = knn_k.tile([P, ncpad], F32, tag="d2")
                for i in range(ncc):
                    a, b = i * 512, min((i + 1) * 512, n_coarse)
                    dps = tps()
                    nc.tensor.matmul(out=dps[:, : b - a], lhsT=lhsT3[:],
                                     rhs=coarse3[:, a:b], start=True, stop=True)
                    nc.vector.tensor_scalar(out=d2[:, a:b], in0=dps[:, : b - a],
                                            scalar1=fsqneg[:, t:t + 1],
                                            scalar2=None, op0=ALU.add)
                vals = knn_k.tile([P, 8], F32, tag="vals")
                nc.vector.max(out=vals[:], in_=d2[:, 0:n_coarse])
                nc.vector.max_index(out=idx_buf[:, t, :], in_max=vals[:],
                                    in_values=d2[:, 0:n_coarse])
                nc.vector.tensor_copy(out=idx3[:, t, :],
                                      in_=idx_buf[:, t, 0:3])
                wv = knn_k.tile([P, 3], F32, tag="wv")
                nc.vector.tensor_scalar(out=wv[:], in0=vals[:, 0:3],
                                        scalar1=-1.0, scalar2=1e-16,
                                        op0=ALU.mult, op1=ALU.max)
                nc.vector.reciprocal(out=wv[:], in_=wv[:])
                wsum = knn_k.tile([P, 1], F32, tag="wsum")
                nc.vector.tensor_reduce(out=wsum[:], in_=wv[:],
                                        axis=mybir.AxisListType.X, op=ALU.add)
                nc.vector.reciprocal(out=wsum[:], in_=wsum[:])
                nc.vector.tensor_scalar(out=wv_buf[:, t, :], in0=wv[:],
                                        scalar1=wsum[:, 0:1], scalar2=None,
                                        op0=ALU.mult)

            def knn_tiles(ts):
                """Phase 1 for 1-2 contiguous tiles; the q0-pinned yg
                indirect gathers (one offset per partition per call — HW
                ucode limit) are deferred via pend_ind."""
                for t in ts:
                    knn_phase1(t)
                cell = {}

                def mk(j, k3, t, nts):
                    def emit():
                        if j == 0 and k3 == 0:
                            cell["yg"] = knn_k.tile([P, 6, out_dim], F32,
                                                    tag="yg", name="yg")
                        yg = cell["yg"]
                        nc.gpsimd.indirect_dma_start(
                            out=yg[:, 3 * j + k3, :], out_offset=None,
                            in_=ycoarse[:, :],
                            in_offset=IOO(ap=idx3[:, t, k3:k3 + 1], axis=0))
                        if k3 == 2:
                            tmp = knn_k.tile([P, out_dim], F32, tag="tmp")
                            nc.vector.tensor_scalar(
                                out=y3n[:, t, :], in0=yg[:, 3 * j, :],
                                scalar1=wv_buf[:, t, 0:1], scalar2=None,
                                op0=ALU.mult)
                            for k in (1, 2):
                                nc.vector.tensor_scalar(
                                    out=tmp[:], in0=yg[:, 3 * j + k, :],
                                    scalar1=wv_buf[:, t, k:k + 1],
                                    scalar2=None, op0=ALU.mult)
                                nc.vector.tensor_tensor(
                                    out=y3n[:, t, :], in0=y3n[:, t, :],
                                    in1=tmp[:], op=ALU.add)
                    return emit

                for j, t in enumerate(ts):
                    for k3 in range(3):
                        pend_ind.append((mk(j, k3, t, len(ts)), 1))

            # KNN tiles spread across the first TWO merged loops (phase-2
            # indirects must all drain before the li==1 loop's dense reads y3n)
            half_nt = math.ceil(nt / 2)
            knn_left = [list(range(0, half_nt)), list(range(half_nt, nt))]

            def pop_knn(lst):
                ts = [lst.pop(0)]
                if lst:
                    ts.append(lst.pop(0))
                return ts

            def fire_parts(t, zsh_l, zfull_l, fired):
                """After tile t's dense lands, fire any completed part AG."""
                for pi, (pt0, pt1) in enumerate(part_tiles):
                    if t == pt1 - 1 and pi not in fired:
                        fired.add(pi)
                        nc.gpsimd.collective_compute(
                            "AllGather", ALU.bypass, replica_groups=rg,
                            ins=[zsh_l[pi][:, :]], outs=[zfull_l[pi][:, :]])

            def dense_tile(t, w_sb, zsh_l, y3_fold):
                """z[t] = h[t] @ W (+ y3 wtop), write bf16 to zsh part."""
                tp_ = t * P
                pi = _part_of(t, part_tiles)
                pt0 = part_tiles[pi][0]
                zps = densps()
                for k in range(kc):
                    nc.tensor.matmul(out=zps[:], lhsT=hT[:, k, tp_:tp_ + P],
                                     rhs=w_sb[:, k, :], start=(k == 0),
                                     stop=(k == kc - 1) and not y3_fold)
                if y3_fold:
                    pt3 = tps()
                    nc.tensor.transpose(out=pt3[0:out_dim, 0:P],
                                        in_=y3n[:, t, :], identity=iden32[:])
                    y3t_T = smallp.tile([out_dim, P], BF16, tag="y3t_T")
                    nc.vector.tensor_copy(out=y3t_T[:], in_=pt3[0:out_dim, 0:P])
                    nc.tensor.matmul(out=zps[:], lhsT=y3t_T[:],
                                     rhs=wtop_sb[:, :], start=False, stop=True)
                z_sb = zp.tile([P, hid], BF16, tag="z_sb")
                nc.scalar.activation(out=z_sb[:], in_=zps[:], func=AF.Copy)
                rw = (t - pt0) * P
                nc.sync.dma_start(out=zsh_l[pi][rw:rw + P, :], in_=z_sb[:])

            # --- L0: q = A h0 (6-wide); h1 = relu(W0^T q + b0); dense W1 ---
            w0_sb = main.tile([6, hid], BF16, tag="w0_sb")
            nc.sync.dma_start(out=w0_sb[:], in_=w0[:, :])
            b0_sb = load_b_mid(b0)
            w1_sb = load_w_mid(w_mid[0])
            fired = set()
            for gi_i, tiles in enumerate(groups):
                msg6 = msg6p.tile([P, maxblk_g, 128], BF16, tag="msg6", name="msg6")
                gather_group(gi_i, msg6, 128, h0p)
                s_sb, c0 = load_S(gi_i)
                for t in tiles:
                    q = accps()
                    blocks = tile_blocks(gi_i, t)
                    for bi, (mb, gb) in enumerate(blocks):
                        S = s_sb[:, (gb - c0) * P:(gb - c0 + 1) * P]
                        nc.tensor.matmul(out=q[0:6, 0:P], lhsT=msg6[:, mb, 0:6],
                                         rhs=S, start=(bi == 0),
                                         stop=(bi == len(blocks) - 1))
                    q_sb = smallp.tile([6, P], BF16, tag="q_sb")
                    nc.vector.tensor_copy(out=q_sb[:], in_=q[0:6, 0:P])
                    tp_ = t * P
                    for jj in range(kc):
                        z0 = densps()
                        nc.tensor.matmul(out=z0[:, 0:P],
                                         lhsT=w0_sb[:, jj * P:(jj + 1) * P],
                                         rhs=q_sb[:], start=True, stop=True)
                        nc.scalar.activation(out=hT[:, jj, tp_:tp_ + P],
                                             in_=z0[:, 0:P], func=AF.Relu,
                                             bias=b0_sb[:, jj:jj + 1])
                    dense_tile(t, w1_sb, zsh[0], False)
                    fire_parts(t, zsh[0], zfull[0], fired)
                if knn_left[0]:
                    knn_tiles(pop_knn(knn_left[0]))
            while knn_left[0]:
                knn_tiles(pop_knn(knn_left[0]))

            # --- L1..L4: scatter(z_li) -> h; dense next (or W5 path) ---
            for li in range(4):
                b_sb = load_b_mid(b_mid[li])
                last = li == 3
                y3_fold = li == 1  # dense after this scatter is ze0 = h3 We0 + y3 wtop
                if not last:
                    wn_sb = load_w_mid(w_mid[li + 1])
                else:
                    w5_sb = main.tile([P, kc, out_dim], BF16, tag="w5_sb")
                    nc.sync.dma_start(
                        out=w5_sb[:], in_=w5[:, :].rearrange("(k p) o -> p k o", p=P))
                    z5w_pp = [main.tile([P, 128], BF16, tag=f"z5wpp{i}",
                                        name="z5wpp") for i in range(2)]
                    for mm_ in z5w_pp:
                        nc.vector.memset(mm_[:], 0.0)
                knn_w = knn_left[li + 1] if li + 1 < len(knn_left) else []
                if li == 1:
                    flush_pending_q0()  # y3n needed by this loop's dense
                fired = set()
                for gi_i, tiles in enumerate(groups):
                    msg = msgp.tile([P, maxblk_g, hid], BF16, tag="msg", name="msg")
                    gather_group(gi_i, msg, hid, zfull[li])
                    s_sb, c0 = load_S(gi_i)
                    for t in tiles:
                        tp_ = t * P
                        hps = accps()
                        blocks = tile_blocks(gi_i, t)
                        nb = len(blocks)
                        for cc in range(kc):
                            for bi, (mb, gb) in enumerate(blocks):
                                nc.tensor.matmul(
                                    out=hps[:, cc * P:(cc + 1) * P],
                                    lhsT=msg[:, mb, cc * P:(cc + 1) * P],
                                    rhs=s_sb[:, (gb - c0) * P:(gb - c0 + 1) * P],
                                    start=(bi == 0),
                                    stop=(bi == nb - 1))
                        for cc in range(kc):
                            nc.scalar.activation(out=hT[:, cc, tp_:tp_ + P],
                                                 in_=hps[:, cc * P:(cc + 1) * P],
                                                 func=AF.Relu,
                                                 bias=b_sb[:, cc:cc + 1])
                        if not last:
                            dense_tile(t, wn_sb, zsh[li + 1], y3_fold)
                            fire_parts(t, zsh[li + 1], zfull[li + 1], fired)
                        else:
                            z5ps = densps()
                            for k in range(kc):
                                nc.tensor.matmul(out=z5ps[0:out_dim, 0:P],
                                                 lhsT=w5_sb[:, k, :],
                                                 rhs=hT[:, k, tp_:tp_ + P],
                                                 start=(k == 0),
                                                 stop=(k == kc - 1))
                            z5T_sb = smallp.tile([out_dim, P], F32, tag="z5T_sb")
                            nc.vector.tensor_copy(out=z5T_sb[:],
                                                  in_=z5ps[0:out_dim, 0:P])
                            ptp = tps()
                            nc.tensor.transpose(out=ptp[:, 0:out_dim],
                                                in_=z5T_sb[:],
                                                identity=iden32[0:out_dim,
                                                                0:out_dim])
                            z5_sb = z5w_pp[t % 2]
                            nc.vector.tensor_copy(out=z5_sb[:, 0:out_dim],
                                                  in_=ptp[:, 0:out_dim])
                            pi = _part_of(t, part_tiles)
                            rw = (t - part_tiles[pi][0]) * P
                            nc.sync.dma_start(out=z5sh[pi][rw:rw + P, :],
                                              in_=z5_sb[:])
                            fire_parts(t, z5sh, z5full, fired)
                    if knn_w:
                        knn_tiles(pop_knn(knn_w))
                while knn_w:
                    knn_tiles(pop_knn(knn_w))

            # --- L5: scatter(z5) + b5 -> out ---
            b5_sb = main.tile([out_dim, 1], F32, tag="b5_sb")
            nc.sync.dma_start(out=b5_sb[:], in_=b5[:, None])
            for gi_i, tiles in enumerate(groups):
                msg3 = msg6p.tile([P, maxblk_g, 128], BF16, tag="msg6", name="msg6")
                gather_group(gi_i, msg3, 128, z5full)
                s_sb, c0 = load_S(gi_i)
                for t in tiles:
                    tp_ = t * P
                    ops = accps()
                    blocks = tile_blocks(gi_i, t)
                    for bi, (mb, gb) in enumerate(blocks):
                        S = s_sb[:, (gb - c0) * P:(gb - c0 + 1) * P]
                        nc.tensor.matmul(out=ops[0:out_dim, 0:P],
                                         lhsT=msg3[:, mb, 0:out_dim], rhs=S,
                                         start=(bi == 0),
                                         stop=(bi == len(blocks) - 1))
                    oT = smallp.tile([out_dim, P], F32, tag="oT")
                    nc.vector.tensor_scalar(out=oT[:], in0=ops[0:out_dim, 0:P],
                                            scalar1=b5_sb[:, 0:1], scalar2=None,
                                            op0=ALU.add)
                    po = tps()
                    nc.tensor.transpose(out=po[:, 0:out_dim], in_=oT[:],
                                        identity=iden32[0:out_dim, 0:out_dim])
                    o_sb = smallp.tile([P, out_dim], F32, tag="o_sb")
                    nc.vector.tensor_copy(out=o_sb[:], in_=po[:, 0:out_dim])
                    nc.sync.dma_start(out=y_out[tp_:tp_ + P, :], in_=o_sb[:])

    nc.finalize()
    return nc


# ---------------------------------------------------------------- entry point
def _prepare(inputs, n_fine, n_coarse, hid, out_dim, ncores):
    import ml_dtypes
    x = np.asarray(inputs["x"], np.float32)
    sdf = np.asarray(inputs["sdf"], np.float32)
    coarse_x = np.asarray(inputs["coarse_x"], np.float32)
    coarse_y = np.asarray(inputs["coarse_y"], np.float32)
    edge_index = np.asarray(inputs["edge_index"])

    nt, padsh, layout, edges = _preprocess_edges(edge_index, n_fine, ncores)
    nsh = n_fine // ncores

    h0 = np.zeros((n_fine, 128), np.float32)
    h0[:, 0:5] = x
    h0[:, 5:6] = sdf
    h0parts = [a.astype(ml_dtypes.bfloat16)
               for a in _part_layout_full(h0, nsh, ncores, layout["part_tiles"])]

    xpos = x[:, :2].astype(np.float32)
    xposT = []
    xpos_nm_l = []
    for c in range(ncores):
        xx = np.zeros((2, padsh), np.float32)
        xx[:, :nsh] = xpos[c * nsh:(c + 1) * nsh].T
        xposT.append(xx)
        xn = np.zeros((padsh, 2), np.float32)
        xn[:nsh] = xpos[c * nsh:(c + 1) * nsh]
        xpos_nm_l.append(xn)
    coarseT = np.ascontiguousarray(coarse_x[:, :2].T).astype(np.float32)

    in_maps = []
    for c in range(ncores):
        m = {
            "idx": edges[c]["idx"],
            "sblk": edges[c]["sblk"],
            "xposT": xposT[c], "xpos_nm": xpos_nm_l[c],
            "coarseT": coarseT, "ycoarse": coarse_y,
            "w0": np.asarray(inputs["pre_W0"], np.float32).astype(ml_dtypes.bfloat16),
            "b0": np.asarray(inputs["pre_b0"], np.float32),
            "w1": np.asarray(inputs["pre_W1"], np.float32).astype(ml_dtypes.bfloat16),
            "b1": np.asarray(inputs["pre_b1"], np.float32),
            "w2": np.asarray(inputs["pre_W2"], np.float32).astype(ml_dtypes.bfloat16),
            "b2": np.asarray(inputs["pre_b2"], np.float32),
            "wtop": np.ascontiguousarray(np.asarray(inputs["end_W0"], np.float32)[:out_dim]).astype(ml_dtypes.bfloat16),
            "we0": np.ascontiguousarray(np.asarray(inputs["end_W0"], np.float32)[out_dim:]).astype(ml_dtypes.bfloat16),
            "be0": np.asarray(inputs["end_b0"], np.float32),
            "we1": np.asarray(inputs["end_W1"], np.float32).astype(ml_dtypes.bfloat16),
            "be1": np.asarray(inputs["end_b1"], np.float32),
            "w5": np.asarray(inputs["end_W2"], np.float32).astype(ml_dtypes.bfloat16),
            "b5": np.asarray(inputs["end_b2"], np.float32),
        }
        for pi in range(len(h0parts)):
            m[f"h0p{pi}"] = h0parts[pi]
        in_maps.append(m)
    return nt, padsh, layout, in_maps


def run(inputs, n_fine=N_FINE, n_coarse=N_COARSE, hid=HID, out_dim=OUT,
        ncores=NCORES, sim=False, trace=False):
    nt, padsh, layout, in_maps = _prepare(inputs, n_fine, n_coarse, hid,
                                          out_dim, ncores)
    key = (n_fine, n_coarse, hid, out_dim, ncores, layout["Kkey"],
           layout["Rkey"])
    if key not in _PROGRAM_CACHE:
        _PROGRAM_CACHE[key] = build_program(n_fine, n_coarse, hid, out_dim,
                                            ncores, nt, layout)
    nc = _PROGRAM_CACHE[key]

    nsh = n_fine // ncores
    if sim:
        from concourse.bass_interp import MultiCoreSim
        ms = MultiCoreSim(nc, ncores, num_workers=1)
        for c in range(ncores):
            for k, v in in_maps[c].items():
                ms.cores[c].tensor(k)[:] = v
        ms.simulate()
        outs = [np.array(ms.cores[c].tensor("out")) for c in range(ncores)]
        exec_ns = None
    else:
        from concourse.bass_utils import run_bass_kernel_spmd
        res = run_bass_kernel_spmd(nc, in_maps, list(range(ncores)), trace=trace)
        global _LAST_RESULT
        _LAST_RESULT = res
        outs = [res.results[c]["out"] for c in range(ncores)]
        exec_ns = res.exec_time_ns

    full = np.zeros((n_fine, out_dim), np.float32)
    for c in range(ncores):
        full[c * nsh:(c + 1) * nsh] = outs[c][:nsh]
    return full, exec_ns


def kernel(**inputs):
    out, _ = run(inputs)
    return out

